# revision 5
# baseline (speedup 1.0000x reference)
"""Trainium2 Bass kernel for nn_DifferentiableQUBO.

reference:
    decisions = sigmoid(scores)            # elementwise, N = 16_777_216 f32
    qubo_loss = sum(decisions * decisions) # scalar
    returns (decisions, qubo_loss)

Strategy (data parallel over 8 NeuronCores):
  - scores is split into 8 contiguous shards of 2_097_152 elements.
  - Each core streams its shard through SBUF in [128, 2048] tiles:
      load (HWDGE via SP) -> sigmoid (ACT) -> store (HWDGE via ACT) and,
      in parallel, square+reduce on DVE (tensor_tensor_reduce) into a
      per-partition accumulator.
  - Per-core output: decisions shard + [128,1] partial sums.
  - Host: concatenate shards, sum the 8*128 partials (f64) -> scalar loss.
"""

import os
import sys

import numpy as np

for _p in ("/opt/trn_rl_repo", "/root/.axon_site/_ro/trn_rl_repo"):
    if os.path.isdir(_p) and _p not in sys.path:
        sys.path.insert(0, _p)
        break

import concourse.bacc as bacc
import concourse.mybir as mybir
from concourse.bass_utils import run_bass_kernel_spmd
from concourse.tile import TileContext

N = 16_777_216
NCORES = 8
SHARD = N // NCORES  # 2_097_152
P = 128
F = 2048
NT = SHARD // (P * F)  # 8 tiles of [128, 2048] f32 (1 MiB) per core

_nc_cache = None


def _build():
    nc = bacc.Bacc("TRN2", num_devices=NCORES)
    x = nc.dram_tensor("scores", [SHARD], mybir.dt.float32, kind="ExternalInput")
    d = nc.dram_tensor("decisions", [SHARD], mybir.dt.float32, kind="ExternalOutput")
    partial = nc.dram_tensor("partial", [P, 1], mybir.dt.float32, kind="ExternalOutput")

    x3 = x.rearrange("(n p f) -> n p f", p=P, f=F)
    d3 = d.rearrange("(n p f) -> n p f", p=P, f=F)

    with TileContext(nc) as tc:
        with (
            tc.tile_pool(name="io", bufs=4) as pool,
            tc.tile_pool(name="acc", bufs=1) as accp,
        ):
            acc = accp.tile([P, NT], mybir.dt.float32)
            accsum = accp.tile([P, 1], mybir.dt.float32)
            for i in range(NT):
                xt = pool.tile([P, F], mybir.dt.float32, tag="x")
                # load issued from SP so stores never block loads
                nc.sync.dma_start(out=xt[:], in_=x3[i])
                dt = pool.tile([P, F], mybir.dt.float32, tag="d")
                nc.scalar.activation(
                    out=dt[:], in_=xt[:], func=mybir.ActivationFunctionType.Sigmoid
                )
                # store issued from ACT: data is ready by program order there
                nc.scalar.dma_start(out=d3[i], in_=dt[:])
                # acc[:, i] = sum_f dt*dt. The squares reuse xt (its data is
                # dead and ACT is done reading it exactly when dt is ready).
                # Not tensor_tensor_reduce: that ISA op fails at runtime on
                # this toolchain (bisected 2026-08-05).
                nc.vector.tensor_mul(out=xt[:], in0=dt[:], in1=dt[:])
                nc.vector.reduce_sum(
                    out=acc[:, i : i + 1], in_=xt[:], axis=mybir.AxisListType.X
                )
            nc.vector.reduce_sum(out=accsum[:], in_=acc[:], axis=mybir.AxisListType.X)
            nc.sync.dma_start(out=partial[:], in_=accsum[:])
    nc.finalize()
    return nc


def _get_nc():
    global _nc_cache
    if _nc_cache is None:
        _nc_cache = _build()
    return _nc_cache


def run(scores: np.ndarray, trace: bool = False):
    """Run on 8 cores. Returns (decisions, loss, exec_time_ns|None)."""
    scores = np.ascontiguousarray(np.asarray(scores, dtype=np.float32))
    assert scores.shape == (N,), scores.shape
    shards = scores.reshape(NCORES, SHARD)
    in_maps = [{"scores": shards[c]} for c in range(NCORES)]
    res = run_bass_kernel_spmd(
        _get_nc(), in_maps, core_ids=list(range(NCORES)), trace=trace
    )
    decisions = np.concatenate(
        [np.asarray(res.results[c]["decisions"]) for c in range(NCORES)]
    )
    partials = np.stack([np.asarray(res.results[c]["partial"]) for c in range(NCORES)])
    loss = np.float32(partials.astype(np.float64).sum())
    return decisions, loss, res.exec_time_ns


def kernel(scores: np.ndarray, data: np.ndarray = None, **_unused) -> tuple:
    decisions, loss, _ = run(scores, trace=False)
    return decisions, loss
